# revision 1
# baseline (speedup 1.0000x reference)
"""BERT_BiLSTM_CRF loss (CRF NLL) Trainium2 kernel — blocked-mixing version.

Self-contained: kernel(**inputs) takes FULL inputs, shards batch across 8
NeuronCores (128 seqs/core on partitions), returns the scalar mean loss.

Forward scores: the CRF forward recurrence S_t = diag(ef_t) E S_{t-1}
(exp domain, G-centered emissions ef=exp(feat-G), E=exp(trans) over the 7
active states) mixes exponentially fast (Birkhoff contraction ~0.5/step),
so the time axis is cut into NBLK=32 blocks of C=64 steps:
  pass 1 (probe): run the last KPROBE steps of every block in parallel from
    a uniform start -> u_b = block-end direction (error ~0.5^KPROBE).
  pass 2: run all blocks fully in parallel, block b seeded with u_{b-1}
    (block 0 exact init). States renormalized (max=1) at tau=31,63 with
    log-offsets; every step's states DMA'd to DRAM as history.
  Cross-block masses cumsummed with tensor_tensor_scan; per-sequence state
  gathered at t=len-1 by indirect DMA, offsets from a [s,blk,3] table.
Validated vs the exact reference in numpy (bf16: final rel err 1.3e-4).

Gold scores: sum_t feat[t,tag_t] via 7 fused eq-mult-accumulate custom DVE
ops; sum_t trans[tag_t,tag_{t-1}] via exact degree-6 Horner polynomials
(host-solved Vandermonde coefficients passed as an input tensor) evaluated
by chained custom DVE ops, then eq-masked accumulation per row.
"""

import numpy as np

B, T, K = 1024, 2048, 9
NCORES = 8
BL = B // NCORES          # sequences per core (=128 partitions)
KA = 7                    # active states
START, STOP = 7, 8
G = 2.4                   # per-step log growth centering
C = 64                    # block length
NBLK = T // C             # 32 blocks
KPROBE = 4                # probe depth (mixing ~0.5^4)
RN = 32                   # renorm interval inside pass 2
FCH = T // 16             # feats DMA chunk

_CACHE = {}
TRACE = False


def _register_dve_ops():
    import concourse.dve_ops as DO
    from concourse.dve_spec import Spec, Src0, Src1, C0, C1, eq, lower, _spill_c3_to_src1
    from concourse.dve_spec import C3
    from concourse.dve_uop import DveOpSpec
    from concourse.dve_spec import AluOp as SAluOp

    existing = {o.name: o for o in DO.OPS}
    if "ANT_EQMUL_ACC" in existing:
        return existing

    def mk(name, spec, subdim=False):
        op = DO.DveOp(name, spec, subdim, uops_sha={})
        DO.OPS.append(op)
        DO.CUSTOM_DVE_SPECS[name] = spec
        DO._SUB_OPCODE_FOR_NAME[name] = DO._CUSTOM_DVE_ROW_BASE + len(DO.OPS) - 1
        for ver in ("v3", "v4"):
            r = DveOpSpec(name=name, opcode=DO.get_dve_sub_opcode(name),
                          uops=lower(spec, ver=ver), rd1_en=DO.has_src1(spec))
            op.uops_sha[ver] = r.sha(ver)
        return op

    def _eqmul_ref(in0, in1, s0, s1, imm2):
        out = (np.asarray(in0, np.float32) == s0) * np.asarray(in1, np.float32)
        acc = (s1 if isinstance(s1, float) else np.asarray(s1, np.float32)) \
            + out.sum(axis=1, keepdims=True)
        return out, acc

    def _horner_ref(in0, in1, s0, s1, imm2):
        x = np.asarray(in0, np.float32)
        c3 = np.asarray(in1, np.float32)
        return s0 + x * (s1 + x * c3)

    ops = {}
    # accum_out = s1 + sum_n (Src0==s0)*Src1
    ops["ANT_EQMUL_ACC"] = mk(
        "ANT_EQMUL_ACC",
        Spec(body=eq(Src0, C0) * Src1, accum=SAluOp.ADD, accum_init=C1,
             reference=_eqmul_ref))
    # out = s0 + x*(s1 + x*c3) with c3 spilled to in1 (read at element 0)
    ops["ANT_HORNER_TOP"] = mk(
        "ANT_HORNER_TOP",
        Spec(body=_spill_c3_to_src1(C0 + Src0 * (C1 + Src0 * C3)),
             reference=_horner_ref))
    # out = s0 + x*(s1 + x*Src1)
    ops["ANT_HORNER_STEP"] = mk(
        "ANT_HORNER_STEP",
        Spec(body=C0 + Src0 * (C1 + Src0 * Src1),
             reference=_horner_ref))
    return ops


def _build_bass():
    import concourse.bass as bass
    import concourse.bacc as bacc
    import concourse.tile as tile
    import concourse.mybir as mybir

    OPS = _register_dve_ops()

    f32 = mybir.dt.float32
    bf16 = mybir.dt.bfloat16
    i32 = mybir.dt.int32
    AX = mybir.AxisListType
    OP = mybir.AluOpType
    AF = mybir.ActivationFunctionType

    nc = bacc.Bacc()

    feats = nc.dram_tensor("feats", [BL, T, K], f32, kind="ExternalInput")
    tagf = nc.dram_tensor("tagf", [BL, T], f32, kind="ExternalInput")
    lenf = nc.dram_tensor("lenf", [BL, 1], f32, kind="ExternalInput")
    leni = nc.dram_tensor("leni", [BL, 1], i32, kind="ExternalInput")
    trans = nc.dram_tensor("trans", [K, K], f32, kind="ExternalInput")
    # Horner coefficients (host Vandermonde): rows 0..6 = trans[j, x-1],
    # row 7 = trans[STOP, x-1], row 8 = trans[x-1, START]; all in x=tag+1.
    coefs = nc.dram_tensor("coefs", [9, 7], f32, kind="ExternalInput")
    outv = nc.dram_tensor("outv", [BL, 1], f32, kind="ExternalOutput")

    hist_d = nc.dram_tensor("hist_d", [C * BL * NBLK, KA], bf16)
    offs_d = nc.dram_tensor("offs_d", [BL * NBLK * 3, 1], f32)

    iota_t_np = np.arange(T, dtype=np.float32).reshape(1, T)
    c_iota_t = nc.inline_tensor(iota_t_np, "c_iota_t")
    c_s32 = nc.inline_tensor((np.arange(BL, dtype=np.int64) * NBLK)
                             .astype(np.int32).reshape(BL, 1), "c_s32")
    c_s96 = nc.inline_tensor((np.arange(BL, dtype=np.int64) * (NBLK * 3))
                             .astype(np.int32).reshape(BL, 1), "c_s96")
    c_sTm1 = nc.inline_tensor((np.arange(BL, dtype=np.int64) * T - 1)
                              .astype(np.int32).reshape(BL, 1), "c_sTm1")

    with tile.TileContext(nc) as tc:
        import contextlib
        ctx = contextlib.ExitStack()
        with ctx:
            sing = ctx.enter_context(tc.tile_pool(name="sing", bufs=1))
            fpool = ctx.enter_context(tc.tile_pool(name="fpool", bufs=2))
            cpool = ctx.enter_context(tc.tile_pool(name="cpool", bufs=2))
            gpool = ctx.enter_context(tc.tile_pool(name="gpool", bufs=2))
            spool = ctx.enter_context(tc.tile_pool(name="spool", bufs=4))

            # ---------- constants ----------
            negG = sing.tile([BL, 1], f32)
            nc.vector.memset(negG[:], -G)
            transb = sing.tile([BL, K * K], f32)
            nc.gpsimd.dma_start(transb[:], bass.AP(trans, 0, [[0, BL], [1, K * K]]))
            coefb = sing.tile([BL, 63], f32)
            nc.gpsimd.dma_start(coefb[:], bass.AP(coefs, 0, [[0, BL], [1, 63]]))
            iota_t = sing.tile([BL, T], f32)
            nc.gpsimd.dma_start(iota_t[:], bass.AP(c_iota_t, 0, [[0, BL], [1, T]]))
            s32 = sing.tile([BL, 1], i32)
            nc.gpsimd.dma_start(s32[:], c_s32[:, :])
            s96 = sing.tile([BL, 1], i32)
            nc.gpsimd.dma_start(s96[:], c_s96[:, :])
            sTm1 = sing.tile([BL, 1], i32)
            nc.gpsimd.dma_start(sTm1[:], c_sTm1[:, :])
            lenf_sb = sing.tile([BL, 1], f32)
            nc.gpsimd.dma_start(lenf_sb[:], lenf[:, :])
            leni_sb = sing.tile([BL, 1], i32)
            nc.gpsimd.dma_start(leni_sb[:], leni[:, :])
            # probe prefetch after consts: slow strided DMA must not block coefb
            featsp = sing.tile([BL, NBLK, KPROBE, K], f32)
            nc.gpsimd.dma_start(
                featsp[:],
                bass.AP(feats, (C - KPROBE) * K,
                        [[T * K, BL], [C * K, NBLK], [1, KPROBE * K]]))
            efp = sing.tile([BL, NBLK, KPROBE, KA], bf16)
            nc.scalar.activation(efp[:], featsp[:, :, :, 0:KA], AF.Exp,
                                 bias=negG[:, 0:1])

            tagf_sb = sing.tile([BL, T], f32)
            nc.sync.dma_start(tagf_sb[:], tagf[:, :])

            trv = transb[:].rearrange("p (j i) -> p j i", i=K)

            # exp(trans) constants (ACT), downcast bf16
            Ef = sing.tile([BL, KA * KA], f32)
            nc.scalar.activation(Ef[:], trv[:, 0:KA, 0:KA], AF.Exp)
            Eb = sing.tile([BL, KA * KA], bf16)
            nc.gpsimd.tensor_copy(Eb[:], Ef[:])
            E7f = sing.tile([BL, KA], f32)
            nc.scalar.activation(E7f[:], trv[:, 0:KA, START:START + 1], AF.Exp)
            E7b = sing.tile([BL, KA], bf16)
            nc.gpsimd.tensor_copy(E7b[:], E7f[:])
            E8b = sing.tile([BL, KA], bf16)
            E8f = sing.tile([BL, KA], f32)
            nc.scalar.activation(E8f[:], trv[:, STOP:STOP + 1, 0:KA], AF.Exp)
            nc.gpsimd.tensor_copy(E8b[:], E8f[:])
            Ebv = Eb[:].rearrange("p (j i) -> p j i", i=KA)

            # ---------- feats DMA + emissions ----------
            featsb = sing.tile([BL, T, K], f32)
            ef = sing.tile([BL, T * KA], bf16)
            efv = ef[:].rearrange("p (t j) -> p t j", j=KA)
            for c in range(T // FCH):
                t0 = c * FCH
                nc.sync.dma_start(featsb[:, t0:t0 + FCH, :], feats[:, t0:t0 + FCH, :])
                nc.scalar.activation(efv[:, t0:t0 + FCH, :],
                                     featsb[:, t0:t0 + FCH, 0:KA], AF.Exp,
                                     bias=negG[:, 0:1])

            # ---------- tags: tagp1m = (tag+1)*(t<len), bf16 ----------
            maskb = sing.tile([BL, T], bf16)
            nc.vector.tensor_tensor(maskb[:], iota_t[:],
                                    lenf_sb[:].broadcast_to([BL, T]), op=OP.is_lt)
            tagp1m = sing.tile([BL, T], bf16)
            nc.vector.scalar_tensor_tensor(tagp1m[:], tagf_sb[:], 1.0, maskb[:],
                                           op0=OP.add, op1=OP.mult)

            # ---------- gold: trans part (Horner per row j) ----------
            acc = sing.tile([BL, 1], f32)
            nc.vector.memset(acc[:], 0.0)
            prevs = tagp1m[:, 0:T - 1]
            curs = tagp1m[:, 1:T]
            for j in range(KA):
                cj = coefb[:, j * 7: j * 7 + 7]
                h1 = gpool.tile([BL, T], f32, tag="h1")
                nc.vector._custom_dve(OPS["ANT_HORNER_TOP"], out=h1[:, 0:T - 1],
                                      in0=prevs, in1=cj[:, 6:7], s0=cj[:, 4:5],
                                      s1=cj[:, 5:6])
                h2 = gpool.tile([BL, T], f32, tag="h2")
                nc.vector._custom_dve(OPS["ANT_HORNER_STEP"], out=h2[:, 0:T - 1],
                                      in0=prevs, in1=h1[:, 0:T - 1],
                                      s0=cj[:, 2:3], s1=cj[:, 3:4])
                h3 = gpool.tile([BL, T], f32, tag="h3")
                nc.vector._custom_dve(OPS["ANT_HORNER_STEP"], out=h3[:, 0:T - 1],
                                      in0=prevs, in1=h2[:, 0:T - 1],
                                      s0=cj[:, 0:1], s1=cj[:, 1:2])
                acc2 = spool.tile([BL, 1], f32, tag="acc")
                junk = gpool.tile([BL, T], f32, tag="h1")
                nc.vector._custom_dve(OPS["ANT_EQMUL_ACC"], out=junk[:, 0:T - 1],
                                      in0=curs, in1=h3[:, 0:T - 1],
                                      s0=float(j + 1), s1=acc[:],
                                      accum_out=acc2[:])
                acc = acc2

            # ---------- forward pass 1: probe ----------
            NB7 = NBLK * KA
            efblk = ef[:].rearrange("p (b c j) -> p b c j", c=C, j=KA)

            def ef_ap(tau):
                # emissions of local step tau for every block: [p, b, j]
                return efblk[:, :, tau, :]

            rsE = sing.tile([BL, KA], f32)
            nc.vector.tensor_reduce(
                out=rsE[:], in_=Eb[:].rearrange("p (j i) -> p j i", i=KA),
                axis=AX.X, op=OP.add)

            y0p = cpool.tile([BL, NB7], bf16, tag="ynew")
            nc.vector.tensor_tensor(
                y0p[:].rearrange("p (b j) -> p b j", j=KA),
                rsE[:].unsqueeze(1).broadcast_to([BL, NBLK, KA]),
                efp[:, :, 0, :], op=OP.mult)
            y_cur = y0p
            with nc.allow_low_precision(reason="validated: bf16 chain, rel 1e-4"):
                for tau in range(C - KPROBE + 1, C):
                    big = cpool.tile([BL, NBLK, KA, KA], bf16, tag="big")
                    nc.vector.tensor_tensor(
                        big[:],
                        y_cur[:].rearrange("p (b i) -> p b i", i=KA)
                            .unsqueeze(2).broadcast_to([BL, NBLK, KA, KA]),
                        Ebv.unsqueeze(1).broadcast_to([BL, NBLK, KA, KA]),
                        op=OP.mult)
                    red = cpool.tile([BL, NBLK, KA], f32, tag="red")
                    nc.vector.tensor_reduce(out=red[:], in_=big[:], axis=AX.X, op=OP.add)
                    y_new = cpool.tile([BL, NB7], bf16, tag="ynew")
                    nc.vector.tensor_tensor(
                        y_new[:].rearrange("p (b j) -> p b j", j=KA), red[:],
                        efp[:, :, tau - (C - KPROBE), :], op=OP.mult)
                    y_cur = y_new

            # normalize probe output
            mxp = spool.tile([BL, NBLK], f32, tag="mxp")
            nc.vector.tensor_reduce(
                out=mxp[:], in_=y_cur[:].rearrange("p (b j) -> p b j", j=KA),
                axis=AX.X, op=OP.max)
            rcp = spool.tile([BL, NBLK], f32, tag="rcp")
            nc.vector.reciprocal(rcp[:], mxp[:])
            un = sing.tile([BL, NB7], bf16)
            nc.vector.tensor_tensor(
                un[:].rearrange("p (b j) -> p b j", j=KA),
                y_cur[:].rearrange("p (b j) -> p b j", j=KA),
                rcp[:].unsqueeze(2).broadcast_to([BL, NBLK, KA]), op=OP.mult)

            # ---------- gold: feat part ----------
            for j in range(KA):
                acc2 = spool.tile([BL, 1], f32, tag="acc")
                junk = gpool.tile([BL, T], f32, tag="h2")
                nc.vector._custom_dve(OPS["ANT_EQMUL_ACC"], out=junk[:],
                                      in0=tagp1m[:, 0:T], in1=featsb[:, :, j],
                                      s0=float(j + 1), s1=acc[:],
                                      accum_out=acc2[:])
                acc = acc2

            # ---------- gold: t0 term trans[tag_0, START] (coef row 8) ----------
            x0 = tagp1m[:, 0:1]
            c8 = coefb[:, 56:63]
            p1 = spool.tile([BL, 1], f32, tag="p1")
            nc.vector._custom_dve(OPS["ANT_HORNER_TOP"], out=p1[:], in0=x0,
                                  in1=c8[:, 6:7], s0=c8[:, 4:5], s1=c8[:, 5:6])
            p2 = spool.tile([BL, 1], f32, tag="p2")
            nc.vector._custom_dve(OPS["ANT_HORNER_STEP"], out=p2[:], in0=x0,
                                  in1=p1[:], s0=c8[:, 2:3], s1=c8[:, 3:4])
            t0p = spool.tile([BL, 1], f32, tag="t0p")
            nc.vector._custom_dve(OPS["ANT_HORNER_STEP"], out=t0p[:], in0=x0,
                                  in1=p2[:], s0=c8[:, 0:1], s1=c8[:, 1:2])

            # ---------- gold: last term trans[STOP, tag_last] ----------
            idxT = spool.tile([BL, 1], i32, tag="idxT")
            nc.gpsimd.tensor_tensor(idxT[:], sTm1[:], leni_sb[:], op=OP.add)
            tgl = spool.tile([BL, 1], f32, tag="tgl")
            nc.gpsimd.indirect_dma_start(
                out=tgl[:], out_offset=None,
                in_=bass.AP(tagf, 0, [[1, BL * T], [1, 1]]),
                in_offset=bass.IndirectOffsetOnAxis(ap=idxT[:, 0:1], axis=0))
            f1 = spool.tile([BL, 1], f32, tag="f1")
            nc.vector.memset(f1[:], 1.0)
            xl = spool.tile([BL, 1], f32, tag="xl")
            nc.vector.tensor_tensor(xl[:], tgl[:], f1[:], op=OP.add)
            c7r = coefb[:, 49:56]
            q1 = spool.tile([BL, 1], f32, tag="q1")
            nc.vector._custom_dve(OPS["ANT_HORNER_TOP"], out=q1[:], in0=xl[:],
                                  in1=c7r[:, 6:7], s0=c7r[:, 4:5], s1=c7r[:, 5:6])
            q2 = spool.tile([BL, 1], f32, tag="q2")
            nc.vector._custom_dve(OPS["ANT_HORNER_STEP"], out=q2[:], in0=xl[:],
                                  in1=q1[:], s0=c7r[:, 2:3], s1=c7r[:, 3:4])
            lastp = spool.tile([BL, 1], f32, tag="lastp")
            nc.vector._custom_dve(OPS["ANT_HORNER_STEP"], out=lastp[:], in0=xl[:],
                                  in1=q2[:], s0=c7r[:, 0:1], s1=c7r[:, 1:2])

            # ---------- gather indices ----------
            li0 = spool.tile([BL, 1], i32, tag="li0")
            cm1 = spool.tile([BL, 1], i32, tag="cm1")
            nc.vector.memset(cm1[:], -1)
            nc.vector.tensor_tensor(li0[:], leni_sb[:], cm1[:], op=OP.add)
            c6i = spool.tile([BL, 1], i32, tag="c6i")
            nc.vector.memset(c6i[:], 6)
            bstar = spool.tile([BL, 1], i32, tag="bstar")
            nc.vector.tensor_tensor(bstar[:], li0[:], c6i[:],
                                    op=OP.logical_shift_right)
            b64 = spool.tile([BL, 1], i32, tag="b64")
            nc.vector.tensor_tensor(b64[:], bstar[:], c6i[:],
                                    op=OP.logical_shift_left)
            taus = spool.tile([BL, 1], i32, tag="taus")
            nc.vector.tensor_tensor(taus[:], li0[:], b64[:], op=OP.subtract)
            c12 = spool.tile([BL, 1], i32, tag="c12")
            nc.vector.memset(c12[:], 12)
            t4096 = spool.tile([BL, 1], i32, tag="t4096")
            nc.vector.tensor_tensor(t4096[:], taus[:], c12[:],
                                    op=OP.logical_shift_left)
            idxA = spool.tile([BL, 1], i32, tag="idxA")
            nc.vector.tensor_tensor(idxA[:], t4096[:], s32[:], op=OP.add)
            nc.vector.tensor_tensor(idxA[:], idxA[:], bstar[:], op=OP.add)
            # nev = (taus+1)>>5 ; idxB = s96 + 3*bstar + nev
            cp1 = spool.tile([BL, 1], i32, tag="cp1")
            nc.vector.memset(cp1[:], 1)
            tp1 = spool.tile([BL, 1], i32, tag="tp1")
            nc.vector.tensor_tensor(tp1[:], taus[:], cp1[:], op=OP.add)
            c5i = spool.tile([BL, 1], i32, tag="c5i")
            nc.vector.memset(c5i[:], 5)
            nev = spool.tile([BL, 1], i32, tag="nev")
            nc.vector.tensor_tensor(nev[:], tp1[:], c5i[:],
                                    op=OP.logical_shift_right)
            b3 = spool.tile([BL, 1], i32, tag="b3")
            nc.vector.tensor_tensor(b3[:], bstar[:], bstar[:], op=OP.add)
            nc.vector.tensor_tensor(b3[:], b3[:], bstar[:], op=OP.add)
            idxB = spool.tile([BL, 1], i32, tag="idxB")
            nc.vector.tensor_tensor(idxB[:], s96[:], b3[:], op=OP.add)
            nc.vector.tensor_tensor(idxB[:], idxB[:], nev[:], op=OP.add)

            # ---------- forward pass 2 ----------
            yin = sing.tile([BL, NB7], bf16)
            nc.vector.memset(yin[:, 0:KA], 1.0)
            nc.vector.tensor_copy(yin[:, KA:NB7], un[:, 0:NB7 - KA])

            lam0 = sing.tile([BL, NBLK], f32)
            lam1 = sing.tile([BL, NBLK], f32)

            y_cur = yin
            for tau in range(C):
                big = cpool.tile([BL, NBLK, KA, KA], bf16, tag="big")
                nc.vector.tensor_tensor(
                    big[:],
                    y_cur[:].rearrange("p (b i) -> p b i", i=KA)
                        .unsqueeze(2).broadcast_to([BL, NBLK, KA, KA]),
                    Ebv.unsqueeze(1).broadcast_to([BL, NBLK, KA, KA]),
                    op=OP.mult)
                red = cpool.tile([BL, NBLK, KA], f32, tag="red")
                nc.vector.tensor_reduce(out=red[:], in_=big[:], axis=AX.X, op=OP.add)
                y_new = cpool.tile([BL, NB7], bf16, tag="ynew")
                nc.vector.tensor_tensor(
                    y_new[:].rearrange("p (b j) -> p b j", j=KA), red[:],
                    ef_ap(tau), op=OP.mult)
                if tau == 0:
                    # block 0 exact init: S_0 = exp(trans[j,START]) * ef_0
                    nc.vector.tensor_tensor(
                        y_new[:, 0:KA], E7b[:], ef[:, 0:KA], op=OP.mult)
                if (tau + 1) % RN == 0:
                    kre = (tau + 1) // RN - 1
                    mx = spool.tile([BL, NBLK], f32, tag="mx")
                    nc.vector.tensor_reduce(
                        out=mx[:], in_=y_new[:].rearrange("p (b j) -> p b j", j=KA),
                        axis=AX.X, op=OP.max)
                    rc = spool.tile([BL, NBLK], f32, tag="rc")
                    nc.vector.reciprocal(rc[:], mx[:])
                    yr = cpool.tile([BL, NB7], bf16, tag="ynew")
                    nc.vector.tensor_tensor(
                        yr[:].rearrange("p (b j) -> p b j", j=KA),
                        y_new[:].rearrange("p (b j) -> p b j", j=KA),
                        rc[:].unsqueeze(2).broadcast_to([BL, NBLK, KA]), op=OP.mult)
                    y_new = yr
                    lam = lam0 if kre == 0 else lam1
                    nc.scalar.activation(lam[:], mx[:], AF.Ln)
                    if kre == 1:
                        nc.gpsimd.tensor_tensor(lam1[:], lam1[:], lam0[:], op=OP.add)
                nc.sync.dma_start(
                    bass.AP(hist_d, tau * BL * NB7, [[NB7, BL], [1, NB7]]),
                    y_new[:])
                y_cur = y_new

            # ---------- cross-block mass cumsum + offsets table ----------
            ones32 = sing.tile([BL, NBLK], f32)
            nc.gpsimd.memset(ones32[:], 1.0)
            lsh = sing.tile([BL, NBLK], f32)
            nc.gpsimd.memset(lsh[:, 0:1], 0.0)
            nc.gpsimd.tensor_copy(lsh[:, 1:NBLK], lam1[:, 0:NBLK - 1])
            Lam = sing.tile([BL, NBLK], f32)
            nc.vector.tensor_tensor_scan(Lam[:], ones32[:], lsh[:], 0.0,
                                         op0=OP.mult, op1=OP.add)
            offs = sing.tile([BL, NBLK * 3], f32)
            ofv = offs[:].rearrange("p (b r) -> p b r", r=3)
            nc.gpsimd.tensor_copy(ofv[:, :, 0:1], Lam[:].unsqueeze(2))
            nc.gpsimd.tensor_tensor(ofv[:, :, 1:2], Lam[:].unsqueeze(2),
                                    lam0[:].unsqueeze(2), op=OP.add)
            nc.gpsimd.tensor_tensor(ofv[:, :, 2:3], Lam[:].unsqueeze(2),
                                    lam1[:].unsqueeze(2), op=OP.add)
            nc.sync.dma_start(
                bass.AP(offs_d, 0, [[NBLK * 3, BL], [1, NBLK * 3]]), offs[:])

            Sg = spool.tile([BL, KA], bf16, tag="Sg")
            nc.gpsimd.indirect_dma_start(
                out=Sg[:], out_offset=None,
                in_=bass.AP(hist_d, 0, [[KA, C * BL * NBLK], [1, KA]]),
                in_offset=bass.IndirectOffsetOnAxis(ap=idxA[:, 0:1], axis=0))
            offg = spool.tile([BL, 1], f32, tag="offg")
            nc.gpsimd.indirect_dma_start(
                out=offg[:], out_offset=None,
                in_=bass.AP(offs_d, 0, [[1, BL * NBLK * 3], [1, 1]]),
                in_offset=bass.IndirectOffsetOnAxis(ap=idxB[:, 0:1], axis=0))

            # ---------- finalize ----------
            junk7 = spool.tile([BL, KA], f32, tag="junk7")
            dotv = spool.tile([BL, 1], f32, tag="dotv")
            nc.vector.tensor_tensor(junk7[:], Sg[:], E8b[:], op=OP.mult)
            nc.vector.tensor_reduce(out=dotv[:, 0:1], in_=junk7[:],
                                    axis=AX.X, op=OP.add)
            lnv = spool.tile([BL, 1], f32, tag="lnv")
            nc.scalar.activation(lnv[:], dotv[:], AF.Ln)
            fwd1 = spool.tile([BL, 1], f32, tag="fwd1")
            nc.vector.tensor_tensor(fwd1[:], lnv[:], offg[:], op=OP.add)
            fwd2 = spool.tile([BL, 1], f32, tag="fwd2")
            nc.vector.scalar_tensor_tensor(fwd2[:], lenf_sb[:], G, fwd1[:],
                                           op0=OP.mult, op1=OP.add)
            g2 = spool.tile([BL, 1], f32, tag="g2")
            nc.vector.tensor_tensor(g2[:], t0p[:], lastp[:], op=OP.add)
            g3 = spool.tile([BL, 1], f32, tag="g3")
            nc.vector.tensor_tensor(g3[:], acc[:], g2[:], op=OP.add)
            res = spool.tile([BL, 1], f32, tag="res")
            nc.vector.tensor_tensor(res[:], fwd2[:], g3[:], op=OP.subtract)
            nc.sync.dma_start(outv[:, :], res[:])

    nc.finalize()
    return nc


def _coefs(transitions):
    tr = np.asarray(transitions, np.float64)
    V = np.vander(np.arange(1, 8, dtype=np.float64), 7, increasing=True)
    rows = [np.linalg.solve(V, tr[j, 0:7]) for j in range(7)]
    rows.append(np.linalg.solve(V, tr[STOP, 0:7]))
    rows.append(np.linalg.solve(V, tr[0:7, START]))
    return np.stack(rows).astype(np.float32)


def kernel(feats, transitions, tags, lengths):
    feats = np.ascontiguousarray(np.asarray(feats, dtype=np.float32))
    transitions = np.ascontiguousarray(np.asarray(transitions, dtype=np.float32))
    tags_f = np.ascontiguousarray(np.asarray(tags).astype(np.float32))
    len_f = np.ascontiguousarray(np.asarray(lengths).astype(np.float32).reshape(B, 1))
    len_i = np.ascontiguousarray(np.asarray(lengths).astype(np.int32).reshape(B, 1))
    coefs = np.ascontiguousarray(_coefs(transitions))

    if "nc" not in _CACHE:
        _CACHE["nc"] = _build_bass()
    nc = _CACHE["nc"]

    from concourse.bass_utils import run_bass_kernel_spmd

    in_maps = []
    for c in range(NCORES):
        sl = slice(c * BL, (c + 1) * BL)
        in_maps.append({
            "feats": feats[sl],
            "tagf": tags_f[sl],
            "lenf": len_f[sl],
            "leni": len_i[sl],
            "trans": transitions,
            "coefs": coefs,
        })
    r = run_bass_kernel_spmd(nc, in_maps, core_ids=list(range(NCORES)),
                             trace=TRACE)
    if TRACE:
        _CACHE["last_result"] = r
    per_seq = np.concatenate([m["outv"].reshape(BL) for m in r.results])
    return np.float32(per_seq.mean(dtype=np.float64))



# revision 16
# speedup vs baseline: 1.7669x; 1.7669x over previous
"""BERT_BiLSTM_CRF loss (CRF NLL) Trainium2 kernel — TensorE-forward version.

Self-contained: kernel(**inputs) takes FULL inputs, shards batch across 8
NeuronCores (128 seqs/core), returns the scalar mean loss.

Forward: the CRF recurrence S_t = diag(ef_t) E S_{t-1} (exp domain,
G-centered emissions) runs on the TensorEngine in a transposed layout:
partitions = 16 groups x 8 states (7 real + 1 zero pad), columns =
4 sets x 128 seqs; blk = set*16+g gives NBLK=64 time blocks of C=32.
Per step: one [128x128]x[128,512] block-diagonal matmul + one DVE mult by
the transposed emissions EFT (built by DMA-XBAR transposes of exp(feats-G)).
Probe pass (last 4 taus, uniform start) provides mass-normalized seeds;
pass 2 runs all blocks in parallel with NO mid renorm (bf16 range is ample);
per-block log-mass lam = ln(sum_j S_end) via ones-blockdiag matmul + Ln;
cross-block exclusive prefix via triangular/full 16x16 matmuls in PSUM.
Every step's state goes to DRAM; per-seq state at t=len-1 and its Lam offset
are fetched by indirect DMA. fwd = ln(S* . E8) + Lam* + G*len.
Validated vs the exact reference in numpy (bf16 chain: loss rel err 8e-7).

Gold scores: sum_t feat[t,tag_t] via 7 fused eq-mult-accumulate custom DVE
ops; sum_t trans[tag_t,tag_{t-1}] via exact degree-6 Horner polynomials
(host-solved Vandermonde coefficients; two degree-3 custom DVE ops per row,
quadratic coeffs inlined as immediates) + eq-masked accumulation per row.
"""

import numpy as np

B, T, K = 1024, 2048, 9
NCORES = 8
BL = B // NCORES          # sequences per core (=128 partitions)
KA = 7                    # active states
JP = 8                    # padded state dim (j=7 is a zero dummy)
START, STOP = 7, 8
G = 2.4                   # per-step log growth centering
C = 32                    # block length
NBLK = T // C             # 64 blocks
NSET = NBLK // 16         # 4 sets; blk = set*16 + g
P = 128                   # partitions in transposed layout: p = g*8 + j
COLS = NSET * BL          # 512 columns: n = set*128 + b
FCH = T // 16             # feats DMA chunk (128 timesteps = 4 blocks)

_CACHE = {}
TRACE = False


def _register_dve_ops():
    import concourse.dve_ops as DO
    from concourse.dve_spec import Spec, Src0, Src1, C0, C1, C2, C3, eq, \
        lower, _spill_c3_to_src1
    from concourse.dve_uop import DveOpSpec
    from concourse.dve_spec import AluOp as SAluOp

    existing = {o.name: o for o in DO.OPS}

    def mk(name, spec, subdim=False):
        if name in existing:
            return existing[name]
        op = DO.DveOp(name, spec, subdim, uops_sha={})
        DO.OPS.append(op)
        DO.CUSTOM_DVE_SPECS[name] = spec
        DO._SUB_OPCODE_FOR_NAME[name] = DO._CUSTOM_DVE_ROW_BASE + len(DO.OPS) - 1
        for ver in ("v3", "v4"):
            r = DveOpSpec(name=name, opcode=DO.get_dve_sub_opcode(name),
                          uops=lower(spec, ver=ver), rd1_en=DO.has_src1(spec))
            op.uops_sha[ver] = r.sha(ver)
        return op

    def _eqmul_ref(in0, in1, s0, s1, imm2):
        out = (np.asarray(in0, np.float32) == s0) * np.asarray(in1, np.float32)
        acc = (s1 if isinstance(s1, float) else np.asarray(s1, np.float32)) \
            + out.sum(axis=1, keepdims=True)
        return out, acc

    def _h3top_ref(in0, in1, s0, s1, imm2):
        x = np.asarray(in0, np.float32)
        c3 = np.asarray(in1, np.float32)
        return s0 + x * (s1 + x * (imm2 + x * c3))

    def _h3step_ref(in0, in1, s0, s1, imm2):
        x = np.asarray(in0, np.float32)
        h = np.asarray(in1, np.float32)
        return s0 + x * (s1 + x * (imm2 + x * h))

    ops = {}
    # accum_out = s1 + sum_n (Src0==s0)*Src1
    ops["ANT_EQMUL_ACC"] = mk(
        "ANT_EQMUL_ACC",
        Spec(body=eq(Src0, C0) * Src1, accum=SAluOp.ADD, accum_init=C1,
             reference=_eqmul_ref))
    # out = s0 + x*(s1 + x*(imm2 + x*c3)) with c3 spilled to in1 (elem 0)
    ops["ANT_H3_TOP"] = mk(
        "ANT_H3_TOP",
        Spec(body=_spill_c3_to_src1(C0 + Src0 * (C1 + Src0 * (C2 + Src0 * C3))),
             reference=_h3top_ref))
    # out = s0 + x*(s1 + x*(imm2 + x*Src1))
    ops["ANT_H3_STEP"] = mk(
        "ANT_H3_STEP",
        Spec(body=C0 + Src0 * (C1 + Src0 * (C2 + Src0 * Src1)),
             reference=_h3step_ref))
    return ops


def _build_bass(coefs_host):
    import concourse.bass as bass
    import concourse.bacc as bacc
    import concourse.tile as tile
    import concourse.mybir as mybir

    OPS = _register_dve_ops()

    f32 = mybir.dt.float32
    bf16 = mybir.dt.bfloat16
    i32 = mybir.dt.int32
    AX = mybir.AxisListType
    OP = mybir.AluOpType
    AF = mybir.ActivationFunctionType

    nc = bacc.Bacc()

    feats = nc.dram_tensor("feats", [BL, T, K], f32, kind="ExternalInput")
    tagf = nc.dram_tensor("tagf", [BL, T], f32, kind="ExternalInput")
    lenf = nc.dram_tensor("lenf", [BL, 1], f32, kind="ExternalInput")
    leni = nc.dram_tensor("leni", [BL, 1], i32, kind="ExternalInput")
    trans = nc.dram_tensor("trans", [K, K], f32, kind="ExternalInput")
    # Horner coefficients (host Vandermonde): rows 0..6 = trans[j, x-1],
    # row 7 = trans[STOP, x-1], row 8 = trans[x-1, START]; all in x=tag+1.
    coefs = nc.dram_tensor("coefs", [9, 7], f32, kind="ExternalInput")
    outv = nc.dram_tensor("outv", [BL, 1], f32, kind="ExternalOutput")

    hist_d = nc.dram_tensor("hist_d", [C * 16 * COLS, 1], f32)
    offs_d = nc.dram_tensor("offs_d", [16 * COLS, 1], f32)

    iota_t_np = np.arange(T, dtype=np.float32).reshape(1, T)
    c_iota_t = nc.inline_tensor(iota_t_np, "c_iota_t")
    c_b32 = nc.inline_tensor(np.arange(BL, dtype=np.int32).reshape(BL, 1),
                             "c_b32")
    c_sTm1 = nc.inline_tensor((np.arange(BL, dtype=np.int64) * T - 1)
                              .astype(np.int32).reshape(BL, 1), "c_sTm1")
    onebd_np = np.zeros((P, 16), np.float32)
    for g in range(16):
        onebd_np[8 * g:8 * g + 8, g] = 1.0
    c_onebd = nc.inline_tensor(onebd_np, "c_onebd")
    bc16_np = np.ascontiguousarray(onebd_np.T)
    c_bc16 = nc.inline_tensor(bc16_np, "c_bc16")
    tri16_np = np.triu(np.ones((16, 16), np.float32), 1)  # [k,m]=1 iff k<m
    c_tri16 = nc.inline_tensor(tri16_np, "c_tri16")
    c_ones16 = nc.inline_tensor(np.ones((16, 16), np.float32), "c_ones16")

    ch = coefs_host  # [9, 7] float, for inline immediates (quadratic coefs)

    with tile.TileContext(nc) as tc:
        import contextlib
        ctx = contextlib.ExitStack()
        with ctx:
            sing = ctx.enter_context(tc.tile_pool(name="sing", bufs=1))
            epool = ctx.enter_context(tc.tile_pool(name="epool", bufs=2))
            cpool = ctx.enter_context(tc.tile_pool(name="cpool", bufs=3))
            gpool = ctx.enter_context(tc.tile_pool(name="gpool", bufs=2))
            spool = ctx.enter_context(tc.tile_pool(name="spool", bufs=4))
            mmps = ctx.enter_context(
                tc.tile_pool(name="mmps", bufs=2, space="PSUM"))
            upps = ctx.enter_context(
                tc.tile_pool(name="upps", bufs=1, space="PSUM"))
            dpps = ctx.enter_context(
                tc.tile_pool(name="dpps", bufs=2, space="PSUM"))

            # ---------- tiny constants ----------
            negG = sing.tile([BL, 1], f32)
            nc.gpsimd.memset(negG[:], -G)
            coefb = sing.tile([BL, 63], f32)
            nc.gpsimd.dma_start(coefb[:], bass.AP(coefs, 0, [[0, BL], [1, 63]]))
            iota_t = sing.tile([BL, T], f32)
            nc.gpsimd.dma_start(iota_t[:], bass.AP(c_iota_t, 0, [[0, BL], [1, T]]))
            b32 = sing.tile([BL, 1], i32)
            nc.gpsimd.dma_start(b32[:], c_b32[:, :])
            sTm1 = sing.tile([BL, 1], i32)
            nc.gpsimd.dma_start(sTm1[:], c_sTm1[:, :])
            lenf_sb = sing.tile([BL, 1], f32)
            nc.gpsimd.dma_start(lenf_sb[:], lenf[:, :])
            leni_sb = sing.tile([BL, 1], i32)
            nc.gpsimd.dma_start(leni_sb[:], leni[:, :])

            # tags: needed early for gold
            tagf_sb = sing.tile([BL, T], f32)
            nc.sync.dma_start(tagf_sb[:], tagf[:, :])

            # E^T small tile: EtT[i, j] = exp(trans[j, i])
            t7 = sing.tile([7, 7], f32)
            nc.gpsimd.dma_start(t7[:], bass.AP(trans, 0, [[1, 7], [9, 7]]))
            EtTe = sing.tile([7, 7], bf16)
            nc.scalar.activation(EtTe[:], t7[:], AF.Exp)
            # E7[j] = exp(trans[j, START]) on partitions j
            t7b = sing.tile([7, 1], f32)
            nc.gpsimd.dma_start(t7b[:], bass.AP(trans, START, [[9, 7], [1, 1]]))
            E7e = sing.tile([7, 1], f32)
            nc.scalar.activation(E7e[:], t7b[:], AF.Exp)
            # E8[j] = exp(trans[STOP, j]) on partitions j
            t8 = sing.tile([7, 1], f32)
            nc.gpsimd.dma_start(t8[:], bass.AP(trans, STOP * K, [[1, 7], [1, 1]]))
            e8f = sing.tile([7, 1], f32)
            nc.scalar.activation(e8f[:], t8[:], AF.Exp)
            e8e = sing.tile([7, 1], bf16)
            nc.gpsimd.tensor_copy(e8e[:], e8f[:])

            # ---------- stationary matrices ----------
            EB = sing.tile([P, P], bf16)          # blockdiag E^T (16 x 8x8)
            nc.vector.memset(EB[:], 0.0)
            for g in range(16):
                nc.gpsimd.dma_start(EB[8 * g:8 * g + 7, 8 * g:8 * g + 7],
                                    EtTe[:, :])
            E8BD = sing.tile([P, 16], bf16)       # [k=(g,j), m=g'] = E8[j]1[g=g']
            nc.gpsimd.memset(E8BD[:], 0.0)
            for g in range(16):
                nc.gpsimd.dma_start(E8BD[8 * g:8 * g + 7, g:g + 1], e8e[:, :])
            ONEBD = sing.tile([P, 16], bf16)      # [k=(g,i), m=g'] = 1[g=g']
            nc.gpsimd.dma_start(ONEBD[:], c_onebd[:, :])
            BC16 = sing.tile([16, P], bf16)       # [k=g, m=(g',j)] = 1[g=g']
            nc.gpsimd.dma_start(BC16[:], c_bc16[:, :])
            TRI16 = sing.tile([16, 16], f32)      # [k, m] = 1[k < m]
            nc.gpsimd.dma_start(TRI16[:], c_tri16[:, :])
            ONES16 = sing.tile([16, 16], f32)
            nc.gpsimd.dma_start(ONES16[:], c_ones16[:, :])
            ones128 = sing.tile([P, 1], bf16)
            nc.gpsimd.memset(ones128[:], 1.0)

            # rsE[(g,j)] = sum_i E[j,i]  (zero on dummy rows)
            rsE_ps = upps.tile([P, 1], f32, tag="rse")
            nc.tensor.matmul(rsE_ps[:], EB[:], ones128[:])

            # ---------- feats DMA + emissions + XBAR transposes ----------
            featsb = sing.tile([BL, T, K], f32)
            # EFT[p=(g,j), tau, set, b] = exp(feats[b, (set*16+g)*32+tau, j] - G)
            EFT = sing.tile([P, C, NSET, BL], bf16)
            for s in range(NSET):
                ef2 = epool.tile([BL, C, 16, JP], bf16, tag="ef2")
                for q in range(4):
                    cidx = s * 4 + q
                    t0 = cidx * FCH
                    eng = nc.sync if (cidx % 2 == 0) else nc.scalar
                    eng.dma_start(featsb[:, t0:t0 + FCH, :],
                                  feats[:, t0:t0 + FCH, :])
                    inap = featsb[:, t0:t0 + FCH, 0:JP] \
                        .rearrange("p (g tau) j -> p tau g j", tau=C)
                    nc.scalar.activation(ef2[:, :, 4 * q:4 * q + 4, :], inap,
                                         AF.Exp, bias=negG[:, 0:1])
                eng = nc.sync if (s % 2 == 0) else nc.scalar
                eng.dma_start(EFT[:, :, s, :],
                              ef2[:].rearrange("p tau g j -> p (tau g j)"),
                              transpose=True)

            def eft(tau):
                return EFT[:, tau, :, :].rearrange("p s b -> p (s b)")

            # ---------- gold: masks ----------
            maskb = sing.tile([BL, T], bf16)
            nc.vector.tensor_tensor(maskb[:], iota_t[:],
                                    lenf_sb[:].broadcast_to([BL, T]), op=OP.is_lt)
            tagp1m = sing.tile([BL, T], bf16)
            nc.vector.scalar_tensor_tensor(tagp1m[:], tagf_sb[:], 1.0, maskb[:],
                                           op0=OP.add, op1=OP.mult)

            # ---------- gold: trans part (2x deg-3 Horner + eqmul per row) ----
            acc = sing.tile([BL, 1], f32)
            nc.vector.memset(acc[:], 0.0)
            junk = sing.tile([BL, T], f32)
            prevs = tagp1m[:, 0:T - 1]
            curs = tagp1m[:, 1:T]
            for j in range(KA):
                cj = coefb[:, j * 7: j * 7 + 7]
                h1 = gpool.tile([BL, T], f32, tag="h1")
                nc.vector._custom_dve(OPS["ANT_H3_TOP"], out=h1[:, 0:T - 1],
                                      in0=prevs, in1=cj[:, 6:7],
                                      s0=cj[:, 3:4], s1=cj[:, 4:5],
                                      imm2=float(ch[j, 5]))
                h2 = gpool.tile([BL, T], f32, tag="h2")
                nc.vector._custom_dve(OPS["ANT_H3_STEP"], out=h2[:, 0:T - 1],
                                      in0=prevs, in1=h1[:, 0:T - 1],
                                      s0=cj[:, 0:1], s1=cj[:, 1:2],
                                      imm2=float(ch[j, 2]))
                acc2 = spool.tile([BL, 1], f32, tag="acc")
                nc.vector._custom_dve(OPS["ANT_EQMUL_ACC"], out=junk[:, 0:T - 1],
                                      in0=curs, in1=h2[:, 0:T - 1],
                                      s0=float(j + 1), s1=acc[:],
                                      accum_out=acc2[:])
                acc = acc2

            # ---------- forward: probe (taus C-4..C-1) ----------
            y0 = cpool.tile([P, COLS], bf16, tag="y")
            nc.vector.tensor_tensor(
                y0[:], eft(C - 4),
                rsE_ps[:, 0:1].broadcast_to([P, COLS]), op=OP.mult)
            y_cur = y0
            for tau in range(C - 3, C):
                mm = mmps.tile([P, COLS], f32, tag="mm")
                nc.tensor.matmul(mm[:], EB[:], y_cur[:])
                y_new = cpool.tile([P, COLS], bf16, tag="y")
                nc.vector.tensor_tensor(y_new[:], mm[:], eft(tau), op=OP.mult)
                y_cur = y_new

            # probe normalization (mass = 1 per (g, n))
            with nc.allow_low_precision(reason="validated: bf16 fwd, rel 1e-4"):
                mps = upps.tile([16, COLS], f32, tag="m16")
                nc.tensor.matmul(mps[:], ONEBD[:], y_cur[:])
                rc = spool.tile([16, COLS], bf16, tag="rc")
                nc.vector.reciprocal(rc[:], mps[:])
                rb = upps.tile([P, COLS], f32, tag="rb")
                nc.tensor.matmul(rb[:], BC16[:], rc[:])
                un = cpool.tile([P, COLS], bf16, tag="y")
                nc.vector.tensor_tensor(un[:], rb[:], y_cur[:], op=OP.mult)

            # ---------- seeds (partition-shifted: DMA, engines can't) -------
            yin = sing.tile([P, COLS], bf16)
            nc.vector.memset(yin[0:8, 0:BL], 0.0)
            nc.sync.dma_start(yin[8:P, :], un[0:P - 8, :])
            for s in range(1, NSET):
                nc.scalar.dma_start(yin[0:8, s * BL:(s + 1) * BL],
                                    un[P - 8:P, (s - 1) * BL:s * BL])

            # ---------- pass 2 ----------
            # per step: y_new = (E y) * ef ; d_tau = E8 . S (per g, n) -> DRAM
            y_cur = yin
            for tau in range(C):
                mm = mmps.tile([P, COLS], f32, tag="mm")
                nc.tensor.matmul(mm[:], EB[:], y_cur[:])
                y_new = cpool.tile([P, COLS], bf16, tag="y")
                nc.vector.tensor_tensor(y_new[:], mm[:], eft(tau), op=OP.mult)
                if tau == 0:
                    # block 0 exact init: S_0 = exp(trans[j,START]) * ef_0
                    nc.vector.tensor_tensor(
                        y_new[0:KA, 0:BL],
                        E7e[:, 0:1].broadcast_to([KA, BL]),
                        EFT[0:KA, 0, 0, :], op=OP.mult)
                dps = dpps.tile([16, COLS], f32, tag="d")
                nc.tensor.matmul(dps[:], E8BD[:], y_new[:])
                d_sb = cpool.tile([16, COLS], f32, tag="dsb")
                nc.scalar.copy(d_sb[:], dps[:])
                eng = nc.sync if (tau % 2 == 0) else nc.scalar
                eng.dma_start(
                    bass.AP(hist_d, tau * 16 * COLS, [[COLS, 16], [1, COLS]]),
                    d_sb[:])
                y_cur = y_new

            # ---------- lam + exclusive prefix (Lam) ----------
            m2 = upps.tile([16, COLS], f32, tag="m16")
            nc.tensor.matmul(m2[:], ONEBD[:], y_cur[:])
            lam_sb = sing.tile([16, COLS], f32)
            nc.scalar.activation(lam_sb[:], m2[:], AF.Ln)
            Lps = upps.tile([16, COLS], f32, tag="lps")
            nc.tensor.matmul(Lps[:], TRI16[:], lam_sb[:],
                             start=True, stop=False, skip_group_check=True)
            pairs = [(sp, s) for s in range(1, NSET) for sp in range(s)]
            for idx, (sp, s) in enumerate(pairs):
                nc.tensor.matmul(Lps[:, s * BL:(s + 1) * BL], ONES16[:],
                                 lam_sb[:, sp * BL:(sp + 1) * BL],
                                 start=False, stop=(idx == len(pairs) - 1),
                                 skip_group_check=True)
            Lam_sb = sing.tile([16, COLS], f32)
            nc.scalar.copy(Lam_sb[:], Lps[:])
            nc.sync.dma_start(bass.AP(offs_d, 0, [[COLS, 16], [1, COLS]]),
                              Lam_sb[:])

            # ---------- gold: feats part ----------
            for j in range(KA):
                acc2 = spool.tile([BL, 1], f32, tag="acc")
                nc.vector._custom_dve(OPS["ANT_EQMUL_ACC"], out=junk[:],
                                      in0=tagp1m[:, 0:T], in1=featsb[:, :, j],
                                      s0=float(j + 1), s1=acc[:],
                                      accum_out=acc2[:])
                acc = acc2

            # ---------- gold: t0 term trans[tag_0, START] (coef row 8) -------
            x0 = tagp1m[:, 0:1]
            c8 = coefb[:, 56:63]
            p1 = spool.tile([BL, 1], f32, tag="p1")
            nc.vector._custom_dve(OPS["ANT_H3_TOP"], out=p1[:], in0=x0,
                                  in1=c8[:, 6:7], s0=c8[:, 3:4], s1=c8[:, 4:5],
                                  imm2=float(ch[8, 5]))
            t0p = spool.tile([BL, 1], f32, tag="t0p")
            nc.vector._custom_dve(OPS["ANT_H3_STEP"], out=t0p[:], in0=x0,
                                  in1=p1[:], s0=c8[:, 0:1], s1=c8[:, 1:2],
                                  imm2=float(ch[8, 2]))

            # ---------- gold: last term trans[STOP, tag_last] ----------
            idxT = spool.tile([BL, 1], i32, tag="idxT")
            nc.gpsimd.tensor_tensor(idxT[:], sTm1[:], leni_sb[:], op=OP.add)
            tgl = spool.tile([BL, 1], f32, tag="tgl")
            nc.gpsimd.indirect_dma_start(
                out=tgl[:], out_offset=None,
                in_=bass.AP(tagf, 0, [[1, BL * T], [1, 1]]),
                in_offset=bass.IndirectOffsetOnAxis(ap=idxT[:, 0:1], axis=0))
            f1 = spool.tile([BL, 1], f32, tag="f1")
            nc.vector.memset(f1[:], 1.0)
            xl = spool.tile([BL, 1], f32, tag="xl")
            nc.vector.tensor_tensor(xl[:], tgl[:], f1[:], op=OP.add)
            c7r = coefb[:, 49:56]
            q1 = spool.tile([BL, 1], f32, tag="q1")
            nc.vector._custom_dve(OPS["ANT_H3_TOP"], out=q1[:], in0=xl[:],
                                  in1=c7r[:, 6:7], s0=c7r[:, 3:4],
                                  s1=c7r[:, 4:5], imm2=float(ch[7, 5]))
            lastp = spool.tile([BL, 1], f32, tag="lastp")
            nc.vector._custom_dve(OPS["ANT_H3_STEP"], out=lastp[:], in0=xl[:],
                                  in1=q1[:], s0=c7r[:, 0:1], s1=c7r[:, 1:2],
                                  imm2=float(ch[7, 2]))

            # ---------- gather indices ----------
            # li0 = len-1; blk = li0>>5; tau = li0&31; s = blk>>4; g = blk&15
            # idxA = tau*8192 + g*512 + s*128 + b ; idxB = g*512 + s*128 + b
            def gp_const(v):
                tl = spool.tile([BL, 1], i32, tag="ic")
                nc.gpsimd.memset(tl[:], v)
                return tl

            cm1 = gp_const(-1)
            c4i = gp_const(4)
            c5i = gp_const(5)
            li0 = spool.tile([BL, 1], i32, tag="li0")
            nc.vector.tensor_tensor(li0[:], leni_sb[:], cm1[:], op=OP.add)
            blkt = spool.tile([BL, 1], i32, tag="blkt")
            nc.vector.tensor_tensor(blkt[:], li0[:], c5i[:],
                                    op=OP.logical_shift_right)
            tmp = spool.tile([BL, 1], i32, tag="tmp")
            nc.vector.tensor_tensor(tmp[:], blkt[:], c5i[:],
                                    op=OP.logical_shift_left)
            taut = spool.tile([BL, 1], i32, tag="taut")
            nc.vector.tensor_tensor(taut[:], li0[:], tmp[:], op=OP.subtract)
            st = spool.tile([BL, 1], i32, tag="st")
            nc.vector.tensor_tensor(st[:], blkt[:], c4i[:],
                                    op=OP.logical_shift_right)
            tmp2 = spool.tile([BL, 1], i32, tag="tmp2")
            nc.vector.tensor_tensor(tmp2[:], st[:], c4i[:],
                                    op=OP.logical_shift_left)
            gt = spool.tile([BL, 1], i32, tag="gt")
            nc.vector.tensor_tensor(gt[:], blkt[:], tmp2[:], op=OP.subtract)

            c13i = gp_const(13)
            c9i = gp_const(9)
            c7i = gp_const(7)
            idxB = spool.tile([BL, 1], i32, tag="idxB")
            nc.vector.tensor_tensor(idxB[:], gt[:], c9i[:],
                                    op=OP.logical_shift_left)
            ts = spool.tile([BL, 1], i32, tag="ts")
            nc.vector.tensor_tensor(ts[:], st[:], c7i[:],
                                    op=OP.logical_shift_left)
            nc.vector.tensor_tensor(idxB[:], idxB[:], ts[:], op=OP.add)
            nc.vector.tensor_tensor(idxB[:], idxB[:], b32[:], op=OP.add)
            idxA = spool.tile([BL, 1], i32, tag="idxA")
            nc.vector.tensor_tensor(idxA[:], taut[:], c13i[:],
                                    op=OP.logical_shift_left)
            nc.vector.tensor_tensor(idxA[:], idxA[:], idxB[:], op=OP.add)

            # ---------- gathers ----------
            dg = spool.tile([BL, 1], f32, tag="dg")
            nc.gpsimd.indirect_dma_start(
                out=dg[:], out_offset=None,
                in_=bass.AP(hist_d, 0, [[1, C * 16 * COLS], [1, 1]]),
                in_offset=bass.IndirectOffsetOnAxis(ap=idxA[:, 0:1], axis=0))
            offg = spool.tile([BL, 1], f32, tag="offg")
            nc.gpsimd.indirect_dma_start(
                out=offg[:], out_offset=None,
                in_=bass.AP(offs_d, 0, [[1, 16 * COLS], [1, 1]]),
                in_offset=bass.IndirectOffsetOnAxis(ap=idxB[:, 0:1], axis=0))

            # ---------- finalize ----------
            lnv = spool.tile([BL, 1], f32, tag="lnv")
            nc.scalar.activation(lnv[:], dg[:], AF.Ln)
            fwd1 = spool.tile([BL, 1], f32, tag="fwd1")
            nc.vector.tensor_tensor(fwd1[:], lnv[:], offg[:], op=OP.add)
            fwd2 = spool.tile([BL, 1], f32, tag="fwd2")
            nc.vector.scalar_tensor_tensor(fwd2[:], lenf_sb[:], G, fwd1[:],
                                           op0=OP.mult, op1=OP.add)
            g2 = spool.tile([BL, 1], f32, tag="g2")
            nc.vector.tensor_tensor(g2[:], t0p[:], lastp[:], op=OP.add)
            g3 = spool.tile([BL, 1], f32, tag="g3")
            nc.vector.tensor_tensor(g3[:], acc[:], g2[:], op=OP.add)
            res = spool.tile([BL, 1], f32, tag="res")
            nc.vector.tensor_tensor(res[:], fwd2[:], g3[:], op=OP.subtract)
            nc.sync.dma_start(outv[:, :], res[:])

    nc.finalize()
    return nc


def _coefs(transitions):
    tr = np.asarray(transitions, np.float64)
    V = np.vander(np.arange(1, 8, dtype=np.float64), 7, increasing=True)
    rows = [np.linalg.solve(V, tr[j, 0:7]) for j in range(7)]
    rows.append(np.linalg.solve(V, tr[STOP, 0:7]))
    rows.append(np.linalg.solve(V, tr[0:7, START]))
    return np.stack(rows).astype(np.float32)


def kernel(feats, transitions, tags, lengths):
    feats = np.ascontiguousarray(np.asarray(feats, dtype=np.float32))
    transitions = np.ascontiguousarray(np.asarray(transitions, dtype=np.float32))
    tags_f = np.ascontiguousarray(np.asarray(tags).astype(np.float32))
    len_f = np.ascontiguousarray(np.asarray(lengths).astype(np.float32).reshape(B, 1))
    len_i = np.ascontiguousarray(np.asarray(lengths).astype(np.int32).reshape(B, 1))
    coefs = np.ascontiguousarray(_coefs(transitions))

    key = ("nc", transitions.tobytes())
    if key not in _CACHE:
        _CACHE[key] = _build_bass(coefs.astype(np.float64))
    nc = _CACHE[key]

    from concourse.bass_utils import run_bass_kernel_spmd

    in_maps = []
    for c in range(NCORES):
        sl = slice(c * BL, (c + 1) * BL)
        in_maps.append({
            "feats": feats[sl],
            "tagf": tags_f[sl],
            "lenf": len_f[sl],
            "leni": len_i[sl],
            "trans": transitions,
            "coefs": coefs,
        })
    r = run_bass_kernel_spmd(nc, in_maps, core_ids=list(range(NCORES)),
                             trace=TRACE)
    if TRACE:
        _CACHE["last_result"] = r
    per_seq = np.concatenate([m["outv"].reshape(BL) for m in r.results])
    return np.float32(per_seq.mean(dtype=np.float64))


# revision 27
# speedup vs baseline: 2.0923x; 1.1842x over previous
"""BERT_BiLSTM_CRF loss (CRF NLL) Trainium2 kernel — TensorE-forward version.

Self-contained: kernel(**inputs) takes FULL inputs, shards batch across 8
NeuronCores (128 seqs/core), returns the scalar mean loss.

Forward: the CRF recurrence S_t = diag(ef_t) E S_{t-1} (exp domain,
G-centered emissions) runs on the TensorEngine in a transposed layout:
partitions = 16 groups x 8 states (7 real + 1 zero pad), columns =
4 sets x 128 seqs; blk = set*16+g gives NBLK=64 time blocks of C=32.
Per step: one [128x128]x[128,512] block-diagonal matmul + one DVE mult by
the transposed emissions EFT (built by DMA-XBAR transposes of exp(feats-G)).
Probe pass (last 4 taus, uniform start) provides mass-normalized seeds;
pass 2 runs all blocks in parallel with NO mid renorm (bf16 range is ample);
per-block log-mass lam = ln(sum_j S_end) via ones-blockdiag matmul + Ln;
cross-block exclusive prefix via triangular/full 16x16 matmuls in PSUM.
Every step's state goes to DRAM; per-seq state at t=len-1 and its Lam offset
are fetched by indirect DMA. fwd = ln(S* . E8) + Lam* + G*len.
Validated vs the exact reference in numpy (bf16 chain: loss rel err 8e-7).

Gold scores: sum_t feat[t,tag_t] via 7 fused eq-mult-accumulate custom DVE
ops; sum_t trans[tag_t,tag_{t-1}] via exact degree-6 Horner polynomials
(host-solved Vandermonde coefficients; two degree-3 custom DVE ops per row,
quadratic coeffs inlined as immediates) + eq-masked accumulation per row.
"""

import numpy as np

B, T, K = 1024, 2048, 9
NCORES = 8
BL = B // NCORES          # sequences per core (=128 partitions)
KA = 7                    # active states
JP = 8                    # padded state dim (j=7 is a zero dummy)
START, STOP = 7, 8
G = 2.4                   # per-step log growth centering
C = 32                    # block length
NBLK = T // C             # 64 blocks
NSET = NBLK // 16         # 4 sets; blk = set*16 + g
P = 128                   # partitions in transposed layout: p = g*8 + j
COLS = NSET * BL          # 512 columns: n = set*128 + b
FCH = T // 16             # feats DMA chunk (128 timesteps = 4 blocks)

_CACHE = {}
TRACE = False


def _register_dve_ops():
    import concourse.dve_ops as DO
    from concourse.dve_spec import Spec, Src0, Src1, C0, C1, C2, C3, eq, \
        lower, _spill_c3_to_src1
    from concourse.dve_uop import DveOpSpec
    from concourse.dve_spec import AluOp as SAluOp

    existing = {o.name: o for o in DO.OPS}

    def mk(name, spec, subdim=False):
        if name in existing:
            return existing[name]
        op = DO.DveOp(name, spec, subdim, uops_sha={})
        DO.OPS.append(op)
        DO.CUSTOM_DVE_SPECS[name] = spec
        DO._SUB_OPCODE_FOR_NAME[name] = DO._CUSTOM_DVE_ROW_BASE + len(DO.OPS) - 1
        for ver in ("v3", "v4"):
            r = DveOpSpec(name=name, opcode=DO.get_dve_sub_opcode(name),
                          uops=lower(spec, ver=ver), rd1_en=DO.has_src1(spec))
            op.uops_sha[ver] = r.sha(ver)
        return op

    def _eqmul_ref(in0, in1, s0, s1, imm2):
        out = (np.asarray(in0, np.float32) == s0) * np.asarray(in1, np.float32)
        acc = (s1 if isinstance(s1, float) else np.asarray(s1, np.float32)) \
            + out.sum(axis=1, keepdims=True)
        return out, acc

    def _h3top_ref(in0, in1, s0, s1, imm2):
        x = np.asarray(in0, np.float32)
        c3 = np.asarray(in1, np.float32)
        return s0 + x * (s1 + x * (imm2 + x * c3))

    def _h3step_ref(in0, in1, s0, s1, imm2):
        x = np.asarray(in0, np.float32)
        h = np.asarray(in1, np.float32)
        return s0 + x * (s1 + x * (imm2 + x * h))

    ops = {}
    # accum_out = s1 + sum_n (Src0==s0)*Src1
    ops["ANT_EQMUL_ACC"] = mk(
        "ANT_EQMUL_ACC",
        Spec(body=eq(Src0, C0) * Src1, accum=SAluOp.ADD, accum_init=C1,
             reference=_eqmul_ref))
    # out = s0 + x*(s1 + x*(imm2 + x*c3)) with c3 spilled to in1 (elem 0)
    ops["ANT_H3_TOP"] = mk(
        "ANT_H3_TOP",
        Spec(body=_spill_c3_to_src1(C0 + Src0 * (C1 + Src0 * (C2 + Src0 * C3))),
             reference=_h3top_ref))
    # out = s0 + x*(s1 + x*(imm2 + x*Src1))
    ops["ANT_H3_STEP"] = mk(
        "ANT_H3_STEP",
        Spec(body=C0 + Src0 * (C1 + Src0 * (C2 + Src0 * Src1)),
             reference=_h3step_ref))
    return ops


def _build_bass(coefs_host):
    import concourse.bass as bass
    import concourse.bacc as bacc
    import concourse.tile as tile
    import concourse.mybir as mybir

    OPS = _register_dve_ops()

    f32 = mybir.dt.float32
    bf16 = mybir.dt.bfloat16
    i32 = mybir.dt.int32
    AX = mybir.AxisListType
    OP = mybir.AluOpType
    AF = mybir.ActivationFunctionType

    nc = bacc.Bacc()

    feats = nc.dram_tensor("feats", [BL, T, K], f32, kind="ExternalInput")
    tagf = nc.dram_tensor("tagf", [BL, T], f32, kind="ExternalInput")
    lenf = nc.dram_tensor("lenf", [BL, 1], f32, kind="ExternalInput")
    leni = nc.dram_tensor("leni", [BL, 1], i32, kind="ExternalInput")
    trans = nc.dram_tensor("trans", [K, K], f32, kind="ExternalInput")
    # Horner coefficients (host Vandermonde): rows 0..6 = trans[j, x-1],
    # row 7 = trans[STOP, x-1], row 8 = trans[x-1, START]; all in x=tag+1.
    coefs = nc.dram_tensor("coefs", [9, 7], f32, kind="ExternalInput")
    outv = nc.dram_tensor("outv", [BL, 1], f32, kind="ExternalOutput")

    hist_d = nc.dram_tensor("hist_d", [(C + 1) * 16 * COLS, 1], bf16)
    offs_d = nc.dram_tensor("offs_d", [16 * COLS, 1], f32)

    iota_t_np = np.arange(T, dtype=np.float32).reshape(1, T)
    c_iota_t = nc.inline_tensor(iota_t_np, "c_iota_t")
    c_b32 = nc.inline_tensor(np.arange(BL, dtype=np.int32).reshape(BL, 1),
                             "c_b32")
    c_sTm1 = nc.inline_tensor((np.arange(BL, dtype=np.int64) * T - 1)
                              .astype(np.int32).reshape(BL, 1), "c_sTm1")
    onebd_np = np.zeros((P, 16), np.float32)
    for g in range(16):
        onebd_np[8 * g:8 * g + 7, g] = 1.0   # exclude dummy row j=7
    c_onebd = nc.inline_tensor(onebd_np, "c_onebd")
    tri16_np = np.triu(np.ones((16, 16), np.float32), 1)  # [k,m]=1 iff k<m
    c_tri16 = nc.inline_tensor(tri16_np, "c_tri16")
    c_ones16 = nc.inline_tensor(np.ones((16, 16), np.float32), "c_ones16")

    ch = coefs_host  # [9, 7] float, for inline immediates (quadratic coefs)

    with tile.TileContext(nc) as tc:
        import contextlib
        ctx = contextlib.ExitStack()
        with ctx:
            sing = ctx.enter_context(tc.tile_pool(name="sing", bufs=1))
            epool = ctx.enter_context(tc.tile_pool(name="epool", bufs=2))
            cpool = ctx.enter_context(tc.tile_pool(name="cpool", bufs=3))
            gpool = ctx.enter_context(tc.tile_pool(name="gpool", bufs=2))
            spool = ctx.enter_context(tc.tile_pool(name="spool", bufs=4))
            mmps = ctx.enter_context(
                tc.tile_pool(name="mmps", bufs=2, space="PSUM"))
            upps = ctx.enter_context(
                tc.tile_pool(name="upps", bufs=1, space="PSUM"))

            # ---------- tiny constants ----------
            negG = sing.tile([BL, 1], f32)
            nc.gpsimd.memset(negG[:], -G)
            coefb = sing.tile([BL, 63], f32)
            nc.gpsimd.dma_start(coefb[:], bass.AP(coefs, 0, [[0, BL], [1, 63]]))
            iota_t = sing.tile([BL, T], f32)
            nc.gpsimd.dma_start(iota_t[:], bass.AP(c_iota_t, 0, [[0, BL], [1, T]]))
            b32 = sing.tile([BL, 1], i32)
            nc.gpsimd.dma_start(b32[:], c_b32[:, :])
            sTm1 = sing.tile([BL, 1], i32)
            nc.gpsimd.dma_start(sTm1[:], c_sTm1[:, :])
            lenf_sb = sing.tile([BL, 1], f32)
            nc.gpsimd.dma_start(lenf_sb[:], lenf[:, :])
            leni_sb = sing.tile([BL, 1], i32)
            nc.gpsimd.dma_start(leni_sb[:], leni[:, :])

            # tags: needed early for gold
            tagf_sb = sing.tile([BL, T], f32)
            nc.sync.dma_start(tagf_sb[:], tagf[:, :])

            # E^T extended tile: EtA[i, j<7] = exp(trans[j, i]),
            # EtA[i, 7] = exp(trans[STOP, i])  (final-score column)
            t7x = sing.tile([7, 8], f32)
            nc.gpsimd.dma_start(t7x[:, 0:7], bass.AP(trans, 0, [[1, 7], [9, 7]]))
            nc.gpsimd.dma_start(t7x[:, 7:8],
                                bass.AP(trans, STOP * K, [[1, 7], [1, 1]]))
            EtA = sing.tile([7, 8], bf16)
            nc.scalar.activation(EtA[:], t7x[:], AF.Exp)
            # E7[j] = exp(trans[j, START]) on partitions j
            t7b = sing.tile([7, 1], f32)
            nc.gpsimd.dma_start(t7b[:], bass.AP(trans, START, [[9, 7], [1, 1]]))
            E7e = sing.tile([7, 1], f32)
            nc.scalar.activation(E7e[:], t7b[:], AF.Exp)

            # ---------- stationary matrices ----------
            # EB blockdiag: col (g,j<7) = E^T block; col (g,7) = E8 (so each
            # matmul's dummy output rows carry d = E8 . S of the PREVIOUS step)
            EB = sing.tile([P, P], bf16)
            nc.vector.memset(EB[:], 0.0)
            for g in range(16):
                nc.gpsimd.dma_start(EB[8 * g:8 * g + 7, 8 * g:8 * g + 8],
                                    EtA[:, :])
            ONEBD = sing.tile([P, 16], bf16)      # [k=(g,i<7), m=g'] = 1[g=g']
            nc.gpsimd.dma_start(ONEBD[:], c_onebd[:, :])
            TRI16 = sing.tile([16, 16], f32)      # [k, m] = 1[k < m]
            nc.gpsimd.dma_start(TRI16[:], c_tri16[:, :])
            ONES16 = sing.tile([16, 16], f32)
            nc.gpsimd.dma_start(ONES16[:], c_ones16[:, :])
            ones128 = sing.tile([P, 1], bf16)
            nc.gpsimd.memset(ones128[:], 1.0)

            # rsE[(g,j)] = sum_i E[j,i]  (zero on dummy rows)
            rsE_ps = upps.tile([P, 1], f32, tag="rse")
            nc.tensor.matmul(rsE_ps[:], EB[:], ones128[:])

            # ---------- feats DMA + emissions + XBAR transposes ----------
            featsb = sing.tile([BL, T, K], f32)
            # EFT[p=(g,j), tau, set, b] = exp(feats[b, (set*16+g)*32+tau, j] - G)
            EFT = sing.tile([P, C, NSET, BL], bf16)
            # feats chunks all on the sync queue (no head-of-line blocking);
            # transposes + everything else on scalar.
            for s in range(NSET):
                ef2 = epool.tile([BL, C, 16, JP], bf16, tag="ef2")
                # dummy-state emissions = 1.0 so matmul d-rows ride unscaled
                nc.vector.memset(ef2[:, :, :, 7:8], 1.0)
                for q in range(4):
                    cidx = s * 4 + q
                    t0 = cidx * FCH
                    nc.sync.dma_start(featsb[:, t0:t0 + FCH, :],
                                      feats[:, t0:t0 + FCH, :])
                    inap = featsb[:, t0:t0 + FCH, 0:KA] \
                        .rearrange("p (g tau) j -> p tau g j", tau=C)
                    nc.scalar.activation(ef2[:, :, 4 * q:4 * q + 4, 0:KA], inap,
                                         AF.Exp, bias=negG[:, 0:1])
                nc.scalar.dma_start(
                    EFT[:, :, s, :],
                    ef2[:].rearrange("p tau g j -> p (tau g j)"),
                    transpose=True)

            def eft(tau):
                return EFT[:, tau, :, :].rearrange("p s b -> p (s b)")

            # ---------- gold: masks ----------
            maskb = sing.tile([BL, T], bf16)
            nc.vector.tensor_tensor(maskb[:], iota_t[:],
                                    lenf_sb[:].broadcast_to([BL, T]), op=OP.is_lt)
            tagp1m = sing.tile([BL, T], bf16)
            nc.vector.scalar_tensor_tensor(tagp1m[:], tagf_sb[:], 1.0, maskb[:],
                                           op0=OP.add, op1=OP.mult)

            # ---------- gold: trans part (2x deg-3 Horner + eqmul per row) ----
            acc = sing.tile([BL, 1], f32)
            nc.vector.memset(acc[:], 0.0)
            junk = sing.tile([BL, T], f32)
            prevs = tagp1m[:, 0:T - 1]
            curs = tagp1m[:, 1:T]
            for j in range(KA):
                cj = coefb[:, j * 7: j * 7 + 7]
                h1 = gpool.tile([BL, T], f32, tag="h1")
                nc.vector._custom_dve(OPS["ANT_H3_TOP"], out=h1[:, 0:T - 1],
                                      in0=prevs, in1=cj[:, 6:7],
                                      s0=cj[:, 3:4], s1=cj[:, 4:5],
                                      imm2=float(ch[j, 5]))
                h2 = gpool.tile([BL, T], f32, tag="h2")
                nc.vector._custom_dve(OPS["ANT_H3_STEP"], out=h2[:, 0:T - 1],
                                      in0=prevs, in1=h1[:, 0:T - 1],
                                      s0=cj[:, 0:1], s1=cj[:, 1:2],
                                      imm2=float(ch[j, 2]))
                acc2 = spool.tile([BL, 1], f32, tag="acc")
                nc.vector._custom_dve(OPS["ANT_EQMUL_ACC"], out=junk[:, 0:T - 1],
                                      in0=curs, in1=h2[:, 0:T - 1],
                                      s0=float(j + 1), s1=acc[:],
                                      accum_out=acc2[:])
                acc = acc2

            # ---------- forward: probe (taus C-4..C-1), unnormalized --------
            HC = COLS // 2

            def eft_h(tau, h):
                return EFT[:, tau, 2 * h:2 * h + 2, :] \
                    .rearrange("p s b -> p (s b)")

            yp = [None, None]
            for h in range(2):
                y0 = cpool.tile([P, HC], bf16, tag=f"y{h}")
                nc.vector.tensor_tensor(
                    y0[:], eft_h(C - 4, h),
                    rsE_ps[:, 0:1].broadcast_to([P, HC]), op=OP.mult)
                yp[h] = y0
            for tau in range(C - 3, C):
                for h in range(2):
                    mm = mmps.tile([P, HC], f32, tag=f"mm{h}")
                    nc.tensor.matmul(mm[:], EB[:], yp[h][:])
                    y_new = cpool.tile([P, HC], bf16, tag=f"y{h}")
                    nc.vector.tensor_tensor(y_new[:], mm[:], eft_h(tau, h),
                                            op=OP.mult)
                    yp[h] = y_new

            # probe block masses: lnm0[g, n] = ln(sum_{j<7} y[(g,j), n])
            mps = upps.tile([16, COLS], f32, tag="m16a")
            for h in range(2):
                nc.tensor.matmul(mps[:, h * HC:(h + 1) * HC], ONEBD[:],
                                 yp[h][:], skip_group_check=True)
            lnm0 = sing.tile([16, COLS], f32)
            nc.scalar.activation(lnm0[:], mps[:], AF.Ln)

            # ---------- seeds (partition-shifted: DMA, engines can't) -------
            yin = sing.tile([P, COLS], bf16)
            nc.vector.memset(yin[0:8, 0:BL], 0.0)
            for h in range(2):
                nc.sync.dma_start(yin[8:P, h * HC:(h + 1) * HC], yp[h][0:P - 8, :])
            for s in range(1, NSET):
                src_h, dst_c = (s - 1) // 2, s * BL
                nc.scalar.dma_start(
                    yin[0:8, dst_c:dst_c + BL],
                    yp[src_h][P - 8:P, ((s - 1) % 2) * BL:((s - 1) % 2) * BL + BL])

            # ---------- pass 2: two independent column-half chains ----------
            # per step: y_new = (E y)*ef. EB's dummy cols carry d = E8.S of
            # the previous step into rows (g,7) (ef pad = 1.0), so hist gets
            # d_{tau-1} from y_new(tau)'s dummy rows; ghost step adds d_{C-1}.
            y_last = [None, None]
            for tau in range(C):
                for h in range(2):
                    y_prev = yin[:, h * HC:(h + 1) * HC] if tau == 0 \
                        else y_last[h][:]
                    mm = mmps.tile([P, HC], f32, tag=f"mm{h}")
                    nc.tensor.matmul(mm[:], EB[:], y_prev)
                    y_new = cpool.tile([P, HC], bf16, tag=f"y{h}")
                    nc.vector.tensor_tensor(y_new[:], mm[:], eft_h(tau, h),
                                            op=OP.mult)
                    if tau == 0 and h == 0:
                        # block 0 exact init: S_0 = exp(trans[j,START]) * ef_0
                        nc.vector.tensor_tensor(
                            y_new[0:KA, 0:BL],
                            E7e[:, 0:1].broadcast_to([KA, BL]),
                            EFT[0:KA, 0, 0, :], op=OP.mult)
                    # y_new(tau) dummy rows hold d_{tau-1} -> region tau
                    drows = y_new[:].rearrange("(g j) n -> g j n", j=8)[:, 7, :]
                    eng = nc.sync if h == 0 else nc.scalar
                    eng.dma_start(
                        bass.AP(hist_d, tau * 16 * COLS + h * HC,
                                [[COLS, 16], [1, HC]]), drows)
                    y_last[h] = y_new
            # ghost step: d_{C-1} shows up in MM(C)'s dummy rows
            for h in range(2):
                mm = mmps.tile([P, HC], f32, tag=f"mm{h}")
                nc.tensor.matmul(mm[:], EB[:], y_last[h][:])
                yg = cpool.tile([P, HC], bf16, tag=f"y{h}")
                nc.scalar.copy(yg[:], mm[:])
                eng = nc.sync if h == 0 else nc.scalar
                gr = yg[:].rearrange("(g j) n -> g j n", j=8)[:, 7, :]
                eng.dma_start(
                    bass.AP(hist_d, C * 16 * COLS + h * HC,
                            [[COLS, 16], [1, HC]]), gr)

            # ---------- delta + exclusive prefix (Lam) ----------
            lam_sb = sing.tile([16, COLS], f32)
            mEp = upps.tile([16, COLS], f32, tag="m16e")
            for h in range(2):
                nc.tensor.matmul(mEp[:, h * HC:(h + 1) * HC], ONEBD[:],
                                 y_last[h][:], skip_group_check=True)
                nc.scalar.activation(lam_sb[:, h * HC:(h + 1) * HC],
                                     mEp[:, h * HC:(h + 1) * HC], AF.Ln)
            dlt = sing.tile([16, COLS], f32)
            nc.vector.tensor_tensor(dlt[:], lam_sb[:], lnm0[:], op=OP.subtract)
            Lps = upps.tile([16, COLS], f32, tag="lps")
            nc.tensor.matmul(Lps[:], TRI16[:], dlt[:],
                             start=True, stop=False, skip_group_check=True)
            pairs = [(sp, s) for s in range(1, NSET) for sp in range(s)]
            for idx, (sp, s) in enumerate(pairs):
                nc.tensor.matmul(Lps[:, s * BL:(s + 1) * BL], ONES16[:],
                                 dlt[:, sp * BL:(sp + 1) * BL],
                                 start=False, stop=(idx == len(pairs) - 1),
                                 skip_group_check=True)
            Lam_sb = sing.tile([16, COLS], f32)
            nc.scalar.copy(Lam_sb[:], Lps[:])
            nc.sync.dma_start(bass.AP(offs_d, 0, [[COLS, 16], [1, COLS]]),
                              Lam_sb[:])

            # ---------- gold: feats part ----------
            for j in range(KA):
                acc2 = spool.tile([BL, 1], f32, tag="acc")
                nc.vector._custom_dve(OPS["ANT_EQMUL_ACC"], out=junk[:],
                                      in0=tagp1m[:, 0:T], in1=featsb[:, :, j],
                                      s0=float(j + 1), s1=acc[:],
                                      accum_out=acc2[:])
                acc = acc2

            # ---------- gold: t0 term trans[tag_0, START] (coef row 8) -------
            x0 = tagp1m[:, 0:1]
            c8 = coefb[:, 56:63]
            p1 = spool.tile([BL, 1], f32, tag="p1")
            nc.vector._custom_dve(OPS["ANT_H3_TOP"], out=p1[:], in0=x0,
                                  in1=c8[:, 6:7], s0=c8[:, 3:4], s1=c8[:, 4:5],
                                  imm2=float(ch[8, 5]))
            t0p = spool.tile([BL, 1], f32, tag="t0p")
            nc.vector._custom_dve(OPS["ANT_H3_STEP"], out=t0p[:], in0=x0,
                                  in1=p1[:], s0=c8[:, 0:1], s1=c8[:, 1:2],
                                  imm2=float(ch[8, 2]))

            # ---------- gold: last term trans[STOP, tag_last] ----------
            idxT = spool.tile([BL, 1], i32, tag="idxT")
            nc.gpsimd.tensor_tensor(idxT[:], sTm1[:], leni_sb[:], op=OP.add)
            tgl = spool.tile([BL, 1], f32, tag="tgl")
            nc.gpsimd.indirect_dma_start(
                out=tgl[:], out_offset=None,
                in_=bass.AP(tagf, 0, [[1, BL * T], [1, 1]]),
                in_offset=bass.IndirectOffsetOnAxis(ap=idxT[:, 0:1], axis=0))
            f1 = spool.tile([BL, 1], f32, tag="f1")
            nc.vector.memset(f1[:], 1.0)
            xl = spool.tile([BL, 1], f32, tag="xl")
            nc.vector.tensor_tensor(xl[:], tgl[:], f1[:], op=OP.add)
            c7r = coefb[:, 49:56]
            q1 = spool.tile([BL, 1], f32, tag="q1")
            nc.vector._custom_dve(OPS["ANT_H3_TOP"], out=q1[:], in0=xl[:],
                                  in1=c7r[:, 6:7], s0=c7r[:, 3:4],
                                  s1=c7r[:, 4:5], imm2=float(ch[7, 5]))
            lastp = spool.tile([BL, 1], f32, tag="lastp")
            nc.vector._custom_dve(OPS["ANT_H3_STEP"], out=lastp[:], in0=xl[:],
                                  in1=q1[:], s0=c7r[:, 0:1], s1=c7r[:, 1:2],
                                  imm2=float(ch[7, 2]))

            # ---------- gather indices ----------
            # li0 = len-1; blk = li0>>5; tau = li0&31; s = blk>>4; g = blk&15
            # idxA = tau*8192 + g*512 + s*128 + b ; idxB = g*512 + s*128 + b
            def gp_const(v):
                tl = spool.tile([BL, 1], i32, tag="ic")
                nc.gpsimd.memset(tl[:], v)
                return tl

            cm1 = gp_const(-1)
            c4i = gp_const(4)
            c5i = gp_const(5)
            li0 = spool.tile([BL, 1], i32, tag="li0")
            nc.vector.tensor_tensor(li0[:], leni_sb[:], cm1[:], op=OP.add)
            blkt = spool.tile([BL, 1], i32, tag="blkt")
            nc.vector.tensor_tensor(blkt[:], li0[:], c5i[:],
                                    op=OP.logical_shift_right)
            tmp = spool.tile([BL, 1], i32, tag="tmp")
            nc.vector.tensor_tensor(tmp[:], blkt[:], c5i[:],
                                    op=OP.logical_shift_left)
            taut = spool.tile([BL, 1], i32, tag="taut")
            nc.vector.tensor_tensor(taut[:], li0[:], tmp[:], op=OP.subtract)
            st = spool.tile([BL, 1], i32, tag="st")
            nc.vector.tensor_tensor(st[:], blkt[:], c4i[:],
                                    op=OP.logical_shift_right)
            tmp2 = spool.tile([BL, 1], i32, tag="tmp2")
            nc.vector.tensor_tensor(tmp2[:], st[:], c4i[:],
                                    op=OP.logical_shift_left)
            gt = spool.tile([BL, 1], i32, tag="gt")
            nc.vector.tensor_tensor(gt[:], blkt[:], tmp2[:], op=OP.subtract)

            c13i = gp_const(13)
            c9i = gp_const(9)
            c7i = gp_const(7)
            idxB = spool.tile([BL, 1], i32, tag="idxB")
            nc.vector.tensor_tensor(idxB[:], gt[:], c9i[:],
                                    op=OP.logical_shift_left)
            ts = spool.tile([BL, 1], i32, tag="ts")
            nc.vector.tensor_tensor(ts[:], st[:], c7i[:],
                                    op=OP.logical_shift_left)
            nc.vector.tensor_tensor(idxB[:], idxB[:], ts[:], op=OP.add)
            nc.vector.tensor_tensor(idxB[:], idxB[:], b32[:], op=OP.add)
            c8k = gp_const(16 * COLS)   # +1 region: d_tau lives at region tau+1
            idxA = spool.tile([BL, 1], i32, tag="idxA")
            nc.vector.tensor_tensor(idxA[:], taut[:], c13i[:],
                                    op=OP.logical_shift_left)
            nc.vector.tensor_tensor(idxA[:], idxA[:], idxB[:], op=OP.add)
            nc.vector.tensor_tensor(idxA[:], idxA[:], c8k[:], op=OP.add)

            # ---------- gathers ----------
            dg = spool.tile([BL, 1], bf16, tag="dg")
            nc.gpsimd.indirect_dma_start(
                out=dg[:], out_offset=None,
                in_=bass.AP(hist_d, 0, [[1, (C + 1) * 16 * COLS], [1, 1]]),
                in_offset=bass.IndirectOffsetOnAxis(ap=idxA[:, 0:1], axis=0))
            offg = spool.tile([BL, 1], f32, tag="offg")
            nc.gpsimd.indirect_dma_start(
                out=offg[:], out_offset=None,
                in_=bass.AP(offs_d, 0, [[1, 16 * COLS], [1, 1]]),
                in_offset=bass.IndirectOffsetOnAxis(ap=idxB[:, 0:1], axis=0))

            # ---------- finalize ----------
            lnv = spool.tile([BL, 1], f32, tag="lnv")
            nc.scalar.activation(lnv[:], dg[:], AF.Ln)
            fwd1 = spool.tile([BL, 1], f32, tag="fwd1")
            nc.vector.tensor_tensor(fwd1[:], lnv[:], offg[:], op=OP.add)
            fwd2 = spool.tile([BL, 1], f32, tag="fwd2")
            nc.vector.scalar_tensor_tensor(fwd2[:], lenf_sb[:], G, fwd1[:],
                                           op0=OP.mult, op1=OP.add)
            g2 = spool.tile([BL, 1], f32, tag="g2")
            nc.vector.tensor_tensor(g2[:], t0p[:], lastp[:], op=OP.add)
            g3 = spool.tile([BL, 1], f32, tag="g3")
            nc.vector.tensor_tensor(g3[:], acc[:], g2[:], op=OP.add)
            res = spool.tile([BL, 1], f32, tag="res")
            nc.vector.tensor_tensor(res[:], fwd2[:], g3[:], op=OP.subtract)
            nc.sync.dma_start(outv[:, :], res[:])

    nc.finalize()
    return nc


def _coefs(transitions):
    tr = np.asarray(transitions, np.float64)
    V = np.vander(np.arange(1, 8, dtype=np.float64), 7, increasing=True)
    rows = [np.linalg.solve(V, tr[j, 0:7]) for j in range(7)]
    rows.append(np.linalg.solve(V, tr[STOP, 0:7]))
    rows.append(np.linalg.solve(V, tr[0:7, START]))
    return np.stack(rows).astype(np.float32)


def kernel(feats, transitions, tags, lengths):
    feats = np.ascontiguousarray(np.asarray(feats, dtype=np.float32))
    transitions = np.ascontiguousarray(np.asarray(transitions, dtype=np.float32))
    tags_f = np.ascontiguousarray(np.asarray(tags).astype(np.float32))
    len_f = np.ascontiguousarray(np.asarray(lengths).astype(np.float32).reshape(B, 1))
    len_i = np.ascontiguousarray(np.asarray(lengths).astype(np.int32).reshape(B, 1))
    coefs = np.ascontiguousarray(_coefs(transitions))

    key = ("nc", transitions.tobytes())
    if key not in _CACHE:
        _CACHE[key] = _build_bass(coefs.astype(np.float64))
    nc = _CACHE[key]

    from concourse.bass_utils import run_bass_kernel_spmd

    in_maps = []
    for c in range(NCORES):
        sl = slice(c * BL, (c + 1) * BL)
        in_maps.append({
            "feats": feats[sl],
            "tagf": tags_f[sl],
            "lenf": len_f[sl],
            "leni": len_i[sl],
            "trans": transitions,
            "coefs": coefs,
        })
    r = run_bass_kernel_spmd(nc, in_maps, core_ids=list(range(NCORES)),
                             trace=TRACE)
    if TRACE:
        _CACHE["last_result"] = r
    per_seq = np.concatenate([m["outv"].reshape(BL) for m in r.results])
    return np.float32(per_seq.mean(dtype=np.float64))
